# revision 29
# baseline (speedup 1.0000x reference)
"""Causal self-attention (B=2, S=2048, HID=1024, 16 heads x 64) on 8 trn2
NeuronCores.

Sharding: data-parallel over batch (cores 0-3 -> batch 0, cores 4-7 ->
batch 1), tensor-parallel over heads (4 heads per core via Wqk/Wv column
slices). Each core computes its 4 heads end-to-end; the [S, S] score
matrix stays core-local.

Per-core layout choices:
  - All matmul operands are bf16 (inputs are cast host-side): the PE
    streams 1 col/cycle at 2.4 GHz and FWL halves LDWEIGHTS time; fp32
    paths measured ~2x slower on HW. PSUM accumulation stays fp32.
  - q, k are produced TRANSPOSED ([head_cols, S]) so score matmuls need
    no on-device transposes; scores are computed transposed ([sk, sq])
    so the P @ v matmul consumes exp(scores) directly from SBUF.
  - v carries an appended ones-column per head; the attention output
    matmul then yields softmax row-sums in an extra partition row for
    free (no max-subtraction is needed: scores are O(5) so exp is safe
    in fp32, and masked entries are zeroed multiplicatively post-exp
    with a DVE multiply against a [128,128] triangle mask input).
  - Heads are processed in pairs: the two K=64 score matmuls sit in PE
    row-groups 0-63 / 64-127 and run concurrently in the array.
  - The P @ v matmuls run 2 chunks BEHIND the score/exp stream, so the
    in-order PE queue never stalls on the ACT engine's exp latency.
  - Inputs arrive host-prepacked in the exact SBUF image as ~13 large
    contiguous DMAs split across the two HWDGE issue queues (Sync +
    Scalar; each dma_start costs ~0.6-1us of issue time, and the front
    is aggregate-HBM-bandwidth-bound, so critical tiles issue first).
  - The triangle masking runs on GPSIMD (IRAM preloaded at t=0), off
    the exp -> P @ v critical path, keeping the DVE queue short: DVE
    reads are what free PSUM tiles for the next score matmuls.
  - Stripes run 0,1,3,2 so the trailing region still has projection
    work as PE filler; pair-tail P @ v + PSUM drain + head finalization
    (transpose + normalize + store) are deferred into the following
    units via a priority queue. With a scratch-tile warmup burst
    bridging the input DMA, the PE holds the HAM clock gate at 8/8 from
    ~14us to the tail (measured: one continuous 123us warm window).
"""
import sys

for _p in ("/opt/trn_rl_repo",):
    if _p not in sys.path:
        sys.path.insert(0, _p)

import numpy as np

B, S, HID = 2, 2048, 1024
NH, HD = 16, 64
NHL = 4            # heads per core
WC = NHL * HD      # 256 local q/k weight cols
VC = NHL * (HD + 1)  # 260 local v cols incl. ones col
NT = S // 128      # 16 key chunks
NA = S // 512      # 4 query stripes
NK = HID // 128    # 8 contraction chunks
LAG = 2            # P @ v trails the score/exp stream by this many chunks

_NC = None


def _build():
    from concourse import bacc, mybir
    from concourse.tile import TileContext

    FP = mybir.dt.float32
    BF = mybir.dt.bfloat16
    Exp = mybir.ActivationFunctionType.Exp

    nc = bacc.Bacc("TRN2", target_bir_lowering=False, debug=False, num_devices=8)

    # all inputs are host-prepacked into the exact SBUF image, so every
    # input DMA is a fully-contiguous identity copy (8KB/partition runs)
    xq_d = [nc.dram_tensor(f"xq{q}", [128, NK * 512], BF, kind="ExternalInput")
            for q in range(4)]
    wq = nc.dram_tensor("wq", [128, NK * WC], BF, kind="ExternalInput")
    wk = nc.dram_tensor("wk", [128, NK * WC], BF, kind="ExternalInput")
    wv = nc.dram_tensor("wv", [128, NK * VC], BF, kind="ExternalInput")
    wvl_d = nc.dram_tensor("wvl", [1, VC], BF, kind="ExternalInput")
    bqk = nc.dram_tensor("bqk", [128, 4], FP, kind="ExternalInput")
    ident_d = nc.dram_tensor("ident", [128, 128], BF, kind="ExternalInput")
    tri_d = nc.dram_tensor("tri", [128, 128], BF, kind="ExternalInput")
    out = nc.dram_tensor("out", [S, WC], FP, kind="ExternalOutput")

    with TileContext(nc) as tc:
        with (
            tc.tile_pool(name="inp", bufs=1) as inp,
            tc.tile_pool(name="ptp", bufs=4) as ptp,
            tc.tile_pool(name="osb", bufs=8) as osb,
            tc.tile_pool(name="rcp", bufs=4) as rcp,
            tc.tile_pool(name="onat", bufs=4) as onp,
            tc.tile_pool(name="G", bufs=3, space="PSUM") as gp,
            tc.tile_pool(name="oT", bufs=2, space="PSUM") as otp,
        ):
            # PE warmup on a zeroed scratch tile (no DMA dependency, so it
            # starts right after the engine preambles): keeps the PE busy
            # through the HAM SHORT window while the input DMA streams, so
            # the projection stream starts at 2.4 GHz instead of 1.2
            scratch = inp.tile([128, 512], BF, name="scratch")
            nc.vector.memset(scratch[:, :], 0.0)
            # preload the GPSIMD ucode IRAM (~6us, hidden in the preamble)
            # so the first in-stream gpsimd mask-multiply doesn't pay it
            gsc = inp.tile([128, 1], BF, name="gsc")
            nc.gpsimd.memset(gsc[:, :], 0.0)
            warm = gp.tile([128, 1024], mybir.dt.float32, tag="G", name="warm")
            for _ in range(32):
                nc.tensor.matmul(warm[:, :512], lhsT=scratch[:, :128],
                                 rhs=scratch[:, :], start=True, stop=True)

            # ---- persistent inputs in SBUF, contiguous identity DMAs
            # split across the two HWDGE issue queues (Sync + Scalar).
            # The front is HBM-bandwidth-bound: issue the critical-path
            # bytes (wq, x quarter 0, wk, wv) before the remaining x
            # quarters, which would otherwise steal bandwidth from them.
            ident = inp.tile([128, 128], BF, name="ident")
            nc.sync.dma_start(ident[:, :], ident_d[:, :])
            tri = inp.tile([128, 128], BF, name="tri")
            nc.scalar.dma_start(tri[:, :], tri_d[:, :])
            # x quarter 0 in two halves, one per issue queue (the front is
            # aggregate-HBM-bound: critical tiles first, big quarters last)
            xq = [[None, None] for _ in range(4)]
            wq_sb = inp.tile([128, NK * WC], BF, name="wq")
            nc.sync.dma_start(wq_sb[:, :], wq[:, :])
            xq[0][1] = inp.tile([128, 4 * 512], BF, name="xq0_1")
            nc.scalar.dma_start(xq[0][1][:, :], xq_d[0][:, 2048:4096])
            xq[0][0] = inp.tile([128, 4 * 512], BF, name="xq0_0")
            nc.sync.dma_start(xq[0][0][:, :], xq_d[0][:, 0:2048])
            wv_sb = inp.tile([128, NK * VC], BF, name="wv")
            nc.scalar.dma_start(wv_sb[:, :], wv[:, :])
            wk_sb = inp.tile([128, NK * WC], BF, name="wk")
            nc.sync.dma_start(wk_sb[:, :], wk[:, :])
            wv_last = inp.tile([1, VC], BF, name="wvl")
            nc.scalar.dma_start(wv_last[:, :], wvl_d[:, :])
            bqk_sb = inp.tile([128, 4], FP, name="bqk")
            nc.scalar.dma_start(bqk_sb[:, :], bqk[:, :])
            for qtr, eng in ((1, nc.scalar), (2, nc.sync), (3, nc.sync)):
                t = inp.tile([128, NK * 512], BF, name=f"xq{qtr}")
                eng.dma_start(t[:, :], xq_d[qtr][:, :])
                xq[qtr][0] = t

            def xk(k, qtr):
                if qtr == 0:
                    t = xq[0][k // 4]
                    return t[:, (k % 4) * 512:(k % 4 + 1) * 512]
                return xq[qtr][0][:, k * 512:(k + 1) * 512]

            # split by S-quarter so interleaved later-quarter projection
            # writes can't false-depend against earlier attention reads
            qT_sb = [[inp.tile([128, 512], BF, name=f"qT{t}_{n}")
                      for n in range(4)] for t in range(2)]
            kT_sb = [[inp.tile([128, 512], BF, name=f"kT{t}_{n}")
                      for n in range(4)] for t in range(2)]
            v_sb = [inp.tile([128, VC], BF, name=f"v{c}") for c in range(NT)]

            # ---- projection emitters ----
            def proj_qk_unit(wt, bcol, dst, t, qtr):
                g = gp.tile([128, 1024], mybir.dt.float32, tag="G", name="g")
                for k in range(NK):
                    nc.tensor.matmul(
                        g[:, :512],
                        lhsT=wt[:, k * WC + t * 128:k * WC + (t + 1) * 128],
                        rhs=xk(k, qtr),
                        start=(k == 0), stop=(k == NK - 1),
                    )
                nc.vector.tensor_scalar_add(
                    dst[t][qtr][:, :], g[:, :512], bqk_sb[:, bcol + t:bcol + t + 1]
                )

            def proj_v_unit(c):
                qtr, cc = divmod(c, 4)
                g = gp.tile([128, 1024], mybir.dt.float32, tag="G", name="g")
                for k in range(NK):
                    nc.tensor.matmul(
                        g[:, :VC],
                        lhsT=xk(k, qtr)[:, cc * 128:(cc + 1) * 128],
                        rhs=wv_sb[:, k * VC:(k + 1) * VC],
                        start=(k == 0), stop=False,
                    )
                nc.tensor.matmul(  # bias row + ones column (K=1)
                    # tri row 0 is all-ones: broadcasts wv_last to all rows
                    g[:, :VC], lhsT=tri[0:1, 0:128], rhs=wv_last[:, :],
                    start=False, stop=True,
                )
                nc.vector.tensor_copy(v_sb[c][:, :], g[:, :VC])

            # ---- attention emitters ----
            # score/exp for ONE key chunk b of a head PAIR:
            # g = [h0-slice | h1-slice], one exp covers both heads
            def score_exp_unit(a, ht, b):
                g = gp.tile([128, 1024], mybir.dt.float32, tag="G", name="g")
                kn, ko = divmod(b * 128, 512)
                # diagonal chunks: columns < off are fully masked -- skip
                # them in the score matmul, the exp, and the P @ v matmul
                off = max(0, (b - 4 * a) * 128)
                for hh in range(2):
                    hb = hh * 64
                    nc.tensor.matmul(
                        g[:, hh * 512 + off:(hh + 1) * 512],
                        lhsT=kT_sb[ht][kn][hb:hb + 64, ko:ko + 128],
                        rhs=qT_sb[ht][a][hb:hb + 64, off:],
                        start=True, stop=True,
                    )
                pt = ptp.tile([128, 1024], BF, tag="pt", name="pt")
                if off:
                    gv = g[:, :].rearrange("p (h w) -> p h w", h=2)[:, :, off:]
                    pv = pt[:, :].rearrange("p (h w) -> p h w", h=2)[:, :, off:]
                    nc.scalar.activation(pv, gv, Exp, scale=HD ** -0.5)
                else:
                    nc.scalar.activation(pt[:, :], g[:, :], Exp, scale=HD ** -0.5)
                if b >= 4 * a:
                    # triangular boundary block: multiplicative mask. On
                    # GPSIMD (otherwise idle) to keep the DVE queue short --
                    # DVE reads are what free PSUM slots for the PE.
                    for hh in range(2):
                        c0 = hh * 512 + off
                        nc.gpsimd.tensor_mul(
                            pt[:, c0:c0 + 128], pt[:, c0:c0 + 128], tri[:, :]
                        )
                return pt

            def av_unit(a, ht, b, nchunks, oTs, pt):
                off = max(0, (b - 4 * a) * 128)
                for hh in range(2):
                    h = 2 * ht + hh
                    nc.tensor.matmul(
                        oTs[hh][:, off:],
                        lhsT=v_sb[b][:, h * 65:(h + 1) * 65],
                        rhs=pt[:, hh * 512 + off:(hh + 1) * 512],
                        start=(b == 0), stop=(b == nchunks - 1),
                    )

            def finish_head(a, ht, hh, oT_sb):
                # transpose + normalize one head: all 4 query blocks go into
                # ONE psum tile, so the G ring is touched once per head (its
                # slot frees only when the DVE reads it -- fewer allocations
                # mean fewer PE stalls on the DVE queue)
                h = 2 * ht + hh
                onat = onat_by_a[a]
                # 66-wide slots keep each bf16 psum write 4-byte aligned
                tr = gp.tile([128, 4 * 66], BF, tag="G", name="tr")
                for c in range(4):
                    nc.tensor.transpose(
                        tr[:, c * 66:c * 66 + HD + 1],
                        oT_sb[:, c * 128:(c + 1) * 128],
                        ident[:HD + 1, :HD + 1],
                    )
                recip = rcp.tile([128, 4], FP, tag="recip", name="recip")
                trv = tr[:, :].rearrange("p (c d) -> p c d", c=4)
                nc.vector.reciprocal(recip[:, :], trv[:, :, HD])
                for c in range(4):
                    nc.vector.tensor_scalar_mul(
                        onat[:, c * WC + h * 64:c * WC + (h + 1) * 64],
                        tr[:, c * 66:c * 66 + HD], recip[:, c:c + 1]
                    )

            # ---- phase 1: the minimum needed by stripe a=0 head pair 0.
            # v0/v1 move into the first filler slots: scores only need q/k,
            # so the exp stream starts ~4us earlier and builds ACT backlog
            proj_qk_unit(wq_sb, 0, qT_sb, 0, 0)
            proj_qk_unit(wk_sb, 2, kT_sb, 0, 0)

            # remaining projection units are doled out between attention
            # units, scheduled (just) before their first consumer, keeping
            # the PE busy while ACT works through the exp stream
            def q_(t, qtr):
                return lambda: proj_qk_unit(wq_sb, 0, qT_sb, t, qtr)

            def k_(t, qtr):
                return lambda: proj_qk_unit(wk_sb, 2, kT_sb, t, qtr)

            def v_(c):
                return lambda: proj_v_unit(c)

            # placement: just-before-first-consumer deadlines, spread so
            # every region keeps the PE slightly ahead of the exp stream.
            # Stripes run 0,1,3,2: the trailing stripe-2 region (24 units)
            # then still has its own q/k projections left as PE filler,
            # where stripe 3 last would leave the PE starved (and the HAM
            # clock gate re-throttling) for its final 16 units.
            # NOTE: stripe 3 consumes ALL kT quarters and v chunks, so only
            # the stripe-2 q projections can be held back for the tail
            def junk_(n):
                # bare weight loads on a scratch tile: PE-array activity the
                # HAM clock gate sees, with no PSUM slot and no readers --
                # keeps K=8/8 through the (HAM-invisible) transpose-heavy
                # tail without perturbing the score/exp pipeline
                def run():
                    for _ in range(n):
                        nc.tensor.ldweights(scratch[:, :128])
                return run

            filler = {
                0: [v_(0)], 1: [v_(1)], 2: [q_(1, 0), v_(2)],
                3: [k_(1, 0), v_(3)],
                4: [q_(0, 1)], 5: [k_(0, 1)], 6: [v_(4)], 7: [v_(5)],
                9: [v_(6)], 11: [v_(7)], 13: [q_(1, 1)], 15: [k_(1, 1)],
                17: [q_(0, 3)], 19: [k_(0, 3)], 21: [v_(8)], 23: [k_(0, 2)],
                25: [v_(9)], 26: [v_(10)], 28: [v_(11)], 30: [v_(12)],
                32: [v_(13)], 34: [v_(14)], 36: [v_(15)], 38: [q_(1, 3)],
                39: [k_(1, 3)], 44: [k_(1, 2)], 50: [q_(0, 2)],
                58: [q_(1, 2)],
            }
            for u in range(64, 80):
                filler.setdefault(u, []).append(junk_(5))

            onat_by_a = {}
            deferred = []          # finish/store closures fed into the stream

            # ---- phases 2+3: attention, software-pipelined ----
            uidx = 0
            for a in (0, 1, 3, 2):
                nchunks = 4 * a + 4
                if a not in onat_by_a:
                    onat_by_a[a] = onp.tile([128, 4 * WC], FP, tag="onat",
                                            name="onat")
                for ht in range(2):
                    oTs = [otp.tile([HD + 1, 512], mybir.dt.float32,
                                    tag="oT", name="oT") for _ in range(2)]
                    pend = []
                    for b in range(nchunks):
                        pend.append((b, score_exp_unit(a, ht, b)))
                        if len(pend) > LAG:
                            bb, pt = pend.pop(0)
                            av_unit(a, ht, bb, nchunks, oTs, pt)
                        for f in filler.get(uidx, ()):
                            f()
                        for _ in range(2 if len(deferred) > 6 else 1):
                            if deferred:
                                deferred.pop(0)()
                        uidx += 1
                    # the tail P @ v matmuls and the oT psum drain flow into
                    # the NEXT pair's units (via the priority end of the
                    # deferred queue): by then the exp stream has caught up,
                    # so the in-order PE queue never waits at pair boundaries
                    bb0, pt0 = pend.pop(0)
                    av_unit(a, ht, bb0, nchunks, oTs, pt0)

                    def tail_av(a_=a, ht_=ht, p_=tuple(pend), n_=nchunks,
                                o_=oTs):
                        for bb, pt in p_:
                            av_unit(a_, ht_, bb, n_, o_, pt)

                    def drain(a_=a, ht_=ht, o_=oTs):
                        for hh in range(2):
                            oT_sb = osb.tile([HD + 1, 512], BF, tag="oTsb",
                                             name="oTsb")
                            nc.vector.tensor_copy(oT_sb[:, :], o_[hh][:, :])
                            deferred.append(
                                (lambda hh_=hh, t_=oT_sb:
                                 finish_head(a_, ht_, hh_, t_)))

                        # store this head pair once its finishes have run
                        def store():
                            nc.sync.dma_start(
                                out[a_ * 512:(a_ + 1) * 512,
                                    ht_ * 128:(ht_ + 1) * 128].rearrange(
                                    "(c p) j -> p c j", p=128),
                                onat_by_a[a_][:, :].rearrange(
                                    "p (c j) -> p c j", c=4)[
                                    :, :, ht_ * 128:(ht_ + 1) * 128],
                            )
                        deferred.append(store)

                    deferred.insert(0, drain)
                    deferred.insert(0, tail_av)
            while deferred:
                deferred.pop(0)()
                junk_(4)()

    nc.compile()
    return nc


def _get_nc():
    global _NC
    if _NC is None:
        _NC = _build()
    return _NC


def make_in_maps(hidden_states, Wqk, bqk, Wv, bv):
    from ml_dtypes import bfloat16

    x = np.asarray(hidden_states, dtype=np.float32)
    Wqk = np.asarray(Wqk, dtype=np.float32)
    bqk = np.asarray(bqk, dtype=np.float32)
    Wv = np.asarray(Wv, dtype=np.float32)
    bv = np.asarray(bv, dtype=np.float32)

    def pack(w):
        # [1024, C] -> SBUF image [128, 8*C] (k-chunk-major columns)
        c = w.shape[1]
        return np.ascontiguousarray(
            w.reshape(NK, 128, c).transpose(1, 0, 2).reshape(128, NK * c)
        ).astype(bfloat16)

    ident = np.eye(128, dtype=bfloat16)
    tri = np.triu(np.ones((128, 128), np.float32)).astype(bfloat16)
    # x quarters as SBUF images: xq[q][p, k*512+j] = x[b].T[k*128+p, q*512+j]
    xqs = []
    for b in range(B):
        xT = x[b].T.reshape(NK, 128, 4, 512)
        xqs.append([np.ascontiguousarray(
            xT[:, :, q, :].transpose(1, 0, 2).reshape(128, NK * 512)
        ).astype(bfloat16) for q in range(4)])
    in_maps = []
    for c in range(8):
        b, ho = c // 4, (c % 4) * NHL
        cols = slice(ho * HD, (ho + NHL) * HD)
        wv_aug = np.zeros((HID, VC), np.float32)
        wvl = np.zeros((1, VC), np.float32)
        for h in range(NHL):
            wv_aug[:, h * 65:h * 65 + HD] = Wv[:, (ho + h) * HD:(ho + h + 1) * HD]
            wvl[0, h * 65:h * 65 + HD] = bv[(ho + h) * HD:(ho + h + 1) * HD]
            wvl[0, h * 65 + HD] = 1.0
        bqk_c = np.stack([bqk[:HID][cols][:128], bqk[:HID][cols][128:],
                          bqk[HID:][cols][:128], bqk[HID:][cols][128:]],
                         axis=1)
        m = {
            "wq": pack(Wqk[:, cols]),
            "wk": pack(Wqk[:, HID:][:, cols]),
            "wv": pack(wv_aug),
            "wvl": wvl.astype(bfloat16),
            "bqk": np.ascontiguousarray(bqk_c.astype(np.float32)),
            "ident": ident,
            "tri": tri,
        }
        for q in range(4):
            m[f"xq{q}"] = xqs[b][q]
        in_maps.append(m)
    return in_maps


def kernel(hidden_states, Wqk, bqk, Wv, bv):
    import time

    from concourse.bass_utils import run_bass_kernel_spmd

    in_maps = make_in_maps(hidden_states, Wqk, bqk, Wv, bv)
    res = None
    for attempt in range(3):
        try:
            res = run_bass_kernel_spmd(_get_nc(), in_maps, list(range(8)))
            break
        except Exception:
            # transient NRT_EXEC_UNIT_UNRECOVERABLE errors have been observed
            # on this fabric; back off and retry
            if attempt == 2:
                raise
            time.sleep(2.0)
    outp = np.empty((B, S, NH * HD), np.float32)
    for c in range(8):
        b, ho = c // 4, (c % 4) * NHL
        outp[b, :, ho * HD:(ho + NHL) * HD] = res.results[c]["out"]
    return outp


# revision 32
# speedup vs baseline: 1.0253x; 1.0253x over previous
"""Causal self-attention (B=2, S=2048, HID=1024, 16 heads x 64) on 8 trn2
NeuronCores.

Sharding: data-parallel over batch (cores 0-3 -> batch 0, cores 4-7 ->
batch 1), tensor-parallel over heads (4 heads per core via Wqk/Wv column
slices). Each core computes its 4 heads end-to-end; the [S, S] score
matrix stays core-local.

Per-core layout choices:
  - All matmul operands are bf16 (inputs are cast host-side): the PE
    streams 1 col/cycle at 2.4 GHz and FWL halves LDWEIGHTS time; fp32
    paths measured ~2x slower on HW. PSUM accumulation stays fp32.
  - q, k are produced TRANSPOSED ([head_cols, S]) so score matmuls need
    no on-device transposes; scores are computed transposed ([sk, sq])
    so the P @ v matmul consumes exp(scores) directly from SBUF.
  - v carries an appended ones-column per head; the attention output
    matmul then yields softmax row-sums in an extra partition row for
    free (no max-subtraction is needed: scores are O(5) so exp is safe
    in fp32, and masked entries are zeroed multiplicatively post-exp
    with a DVE multiply against a [128,128] triangle mask input).
  - Heads are processed in pairs: the two K=64 score matmuls sit in PE
    row-groups 0-63 / 64-127 and run concurrently in the array.
  - The P @ v matmuls run 2 chunks BEHIND the score/exp stream, so the
    in-order PE queue never stalls on the ACT engine's exp latency.
  - Inputs arrive host-prepacked in the exact SBUF image as ~13 large
    contiguous DMAs split across the two HWDGE issue queues (Sync +
    Scalar; each dma_start costs ~0.6-1us of issue time, and the front
    is aggregate-HBM-bandwidth-bound, so critical tiles issue first).
  - The triangle masking runs on GPSIMD (IRAM preloaded at t=0), off
    the exp -> P @ v critical path, keeping the DVE queue short: DVE
    reads are what free PSUM tiles for the next score matmuls.
  - Stripes run 0,1,3,2 so the trailing region still has projection
    work as PE filler; pair-tail P @ v + PSUM drain + head finalization
    (transpose + normalize + store) are deferred into the following
    units via a priority queue. With a scratch-tile warmup burst
    bridging the input DMA, the PE holds the HAM clock gate at 8/8 from
    ~14us to the tail (measured: one continuous 123us warm window).
"""
import sys

for _p in ("/opt/trn_rl_repo",):
    if _p not in sys.path:
        sys.path.insert(0, _p)

import numpy as np

B, S, HID = 2, 2048, 1024
NH, HD = 16, 64
NHL = 4            # heads per core
WC = NHL * HD      # 256 local q/k weight cols
VC = NHL * (HD + 1)  # 260 local v cols incl. ones col
NT = S // 128      # 16 key chunks
NA = S // 512      # 4 query stripes
NK = HID // 128    # 8 contraction chunks
LAG = 2            # P @ v trails the score/exp stream by this many chunks

_NC = None


def _build():
    from concourse import bacc, mybir
    from concourse.tile import TileContext

    FP = mybir.dt.float32
    BF = mybir.dt.bfloat16
    Exp = mybir.ActivationFunctionType.Exp

    nc = bacc.Bacc("TRN2", target_bir_lowering=False, debug=False, num_devices=8)

    # all inputs are host-prepacked into the exact SBUF image, so every
    # input DMA is a fully-contiguous identity copy (8KB/partition runs)
    xq_d = [nc.dram_tensor(f"xq{q}", [128, NK * 512], BF, kind="ExternalInput")
            for q in range(4)]
    wq = nc.dram_tensor("wq", [128, NK * WC], BF, kind="ExternalInput")
    wk = nc.dram_tensor("wk", [128, NK * WC], BF, kind="ExternalInput")
    wv = nc.dram_tensor("wv", [128, NK * VC], BF, kind="ExternalInput")
    wvl_d = nc.dram_tensor("wvl", [1, VC], BF, kind="ExternalInput")
    bqk = nc.dram_tensor("bqk", [128, 4], FP, kind="ExternalInput")
    ident_d = nc.dram_tensor("ident", [128, 128], BF, kind="ExternalInput")
    tri_d = nc.dram_tensor("tri", [128, 128], BF, kind="ExternalInput")
    out = nc.dram_tensor("out", [S, WC], FP, kind="ExternalOutput")

    with TileContext(nc) as tc:
        with (
            tc.tile_pool(name="inp", bufs=1) as inp,
            tc.tile_pool(name="ptp", bufs=4) as ptp,
            tc.tile_pool(name="osb", bufs=8) as osb,
            tc.tile_pool(name="rcp", bufs=4) as rcp,
            tc.tile_pool(name="onat", bufs=4) as onp,
            tc.tile_pool(name="G", bufs=3, space="PSUM") as gp,
            tc.tile_pool(name="oT", bufs=2, space="PSUM") as otp,
        ):
            # PE warmup on a zeroed scratch tile (no DMA dependency, so it
            # starts right after the engine preambles): keeps the PE busy
            # through the HAM SHORT window while the input DMA streams, so
            # the projection stream starts at 2.4 GHz instead of 1.2
            scratch = inp.tile([128, 512], BF, name="scratch")
            nc.vector.memset(scratch[:, :], 0.0)
            # preload the GPSIMD ucode IRAM (~6us, hidden in the preamble)
            # so the first in-stream gpsimd mask-multiply doesn't pay it
            gsc = inp.tile([128, 1], BF, name="gsc")
            nc.gpsimd.memset(gsc[:, :], 0.0)
            warm = gp.tile([128, 1024], mybir.dt.float32, tag="G", name="warm")
            for _ in range(32):
                nc.tensor.matmul(warm[:, :512], lhsT=scratch[:, :128],
                                 rhs=scratch[:, :], start=True, stop=True)

            # ---- persistent inputs in SBUF, contiguous identity DMAs
            # split across the two HWDGE issue queues (Sync + Scalar).
            # The front is HBM-bandwidth-bound: issue the critical-path
            # bytes (wq, x quarter 0, wk, wv) before the remaining x
            # quarters, which would otherwise steal bandwidth from them.
            ident = inp.tile([128, 128], BF, name="ident")
            nc.sync.dma_start(ident[:, :], ident_d[:, :])
            tri = inp.tile([128, 128], BF, name="tri")
            nc.scalar.dma_start(tri[:, :], tri_d[:, :])
            # x quarter 0 in two halves, one per issue queue (the front is
            # aggregate-HBM-bound: critical tiles first, big quarters last)
            xq = [[None, None] for _ in range(4)]
            wq_sb = inp.tile([128, NK * WC], BF, name="wq")
            nc.sync.dma_start(wq_sb[:, :], wq[:, :])
            xq[0][1] = inp.tile([128, 4 * 512], BF, name="xq0_1")
            nc.scalar.dma_start(xq[0][1][:, :], xq_d[0][:, 2048:4096])
            xq[0][0] = inp.tile([128, 4 * 512], BF, name="xq0_0")
            nc.sync.dma_start(xq[0][0][:, :], xq_d[0][:, 0:2048])
            wv_sb = inp.tile([128, NK * VC], BF, name="wv")
            nc.scalar.dma_start(wv_sb[:, :], wv[:, :])
            wk_sb = inp.tile([128, NK * WC], BF, name="wk")
            nc.sync.dma_start(wk_sb[:, :], wk[:, :])
            wv_last = inp.tile([1, VC], BF, name="wvl")
            nc.scalar.dma_start(wv_last[:, :], wvl_d[:, :])
            bqk_sb = inp.tile([128, 4], FP, name="bqk")
            nc.scalar.dma_start(bqk_sb[:, :], bqk[:, :])
            for qtr, eng in ((1, nc.scalar), (2, nc.sync), (3, nc.sync)):
                t = inp.tile([128, NK * 512], BF, name=f"xq{qtr}")
                eng.dma_start(t[:, :], xq_d[qtr][:, :])
                xq[qtr][0] = t

            def xk(k, qtr):
                if qtr == 0:
                    t = xq[0][k // 4]
                    return t[:, (k % 4) * 512:(k % 4 + 1) * 512]
                return xq[qtr][0][:, k * 512:(k + 1) * 512]

            # split by S-quarter so interleaved later-quarter projection
            # writes can't false-depend against earlier attention reads
            qT_sb = [[inp.tile([128, 512], BF, name=f"qT{t}_{n}")
                      for n in range(4)] for t in range(2)]
            kT_sb = [[inp.tile([128, 512], BF, name=f"kT{t}_{n}")
                      for n in range(4)] for t in range(2)]
            v_sb = [inp.tile([128, VC], BF, name=f"v{c}") for c in range(NT)]

            # ---- projection emitters ----
            def proj_qk_unit(wt, bcol, dst, t, qtr):
                g = gp.tile([128, 1024], mybir.dt.float32, tag="G", name="g")
                for k in range(NK):
                    nc.tensor.matmul(
                        g[:, :512],
                        lhsT=wt[:, k * WC + t * 128:k * WC + (t + 1) * 128],
                        rhs=xk(k, qtr),
                        start=(k == 0), stop=(k == NK - 1),
                    )
                nc.vector.tensor_scalar_add(
                    dst[t][qtr][:, :], g[:, :512], bqk_sb[:, bcol + t:bcol + t + 1]
                )

            def proj_v_unit(c):
                qtr, cc = divmod(c, 4)
                g = gp.tile([128, 1024], mybir.dt.float32, tag="G", name="g")
                for k in range(NK):
                    nc.tensor.matmul(
                        g[:, :VC],
                        lhsT=xk(k, qtr)[:, cc * 128:(cc + 1) * 128],
                        rhs=wv_sb[:, k * VC:(k + 1) * VC],
                        start=(k == 0), stop=False,
                    )
                nc.tensor.matmul(  # bias row + ones column (K=1)
                    # tri row 0 is all-ones: broadcasts wv_last to all rows
                    g[:, :VC], lhsT=tri[0:1, 0:128], rhs=wv_last[:, :],
                    start=False, stop=True,
                )
                nc.vector.tensor_copy(v_sb[c][:, :], g[:, :VC])

            # ---- attention emitters ----
            # score/exp for ONE key chunk b of a head PAIR:
            # g = [h0-slice | h1-slice], one exp covers both heads
            def score_exp_unit(a, ht, b):
                g = gp.tile([128, 1024], mybir.dt.float32, tag="G", name="g")
                kn, ko = divmod(b * 128, 512)
                # diagonal chunks: columns < off are fully masked -- skip
                # them in the score matmul, the exp, and the P @ v matmul
                off = max(0, (b - 4 * a) * 128)
                for hh in range(2):
                    hb = hh * 64
                    nc.tensor.matmul(
                        g[:, hh * 512 + off:(hh + 1) * 512],
                        lhsT=kT_sb[ht][kn][hb:hb + 64, ko:ko + 128],
                        rhs=qT_sb[ht][a][hb:hb + 64, off:],
                        start=True, stop=True,
                    )
                pt = ptp.tile([128, 1024], BF, tag="pt", name="pt")
                if off:
                    gv = g[:, :].rearrange("p (h w) -> p h w", h=2)[:, :, off:]
                    pv = pt[:, :].rearrange("p (h w) -> p h w", h=2)[:, :, off:]
                    nc.scalar.activation(pv, gv, Exp, scale=HD ** -0.5)
                else:
                    nc.scalar.activation(pt[:, :], g[:, :], Exp, scale=HD ** -0.5)
                if b >= 4 * a:
                    # triangular boundary block: multiplicative mask. On
                    # GPSIMD (otherwise idle) to keep the DVE queue short --
                    # DVE reads are what free PSUM slots for the PE.
                    for hh in range(2):
                        c0 = hh * 512 + off
                        nc.gpsimd.tensor_mul(
                            pt[:, c0:c0 + 128], pt[:, c0:c0 + 128], tri[:, :]
                        )
                return pt

            def av_unit(a, ht, b, nchunks, oTs, pt):
                off = max(0, (b - 4 * a) * 128)
                for hh in range(2):
                    h = 2 * ht + hh
                    nc.tensor.matmul(
                        oTs[hh][:, off:],
                        lhsT=v_sb[b][:, h * 65:(h + 1) * 65],
                        rhs=pt[:, hh * 512 + off:(hh + 1) * 512],
                        start=(b == 0), stop=(b == nchunks - 1),
                    )

            def finish_head(a, ht, hh, oT_sb):
                # transpose + normalize one head: all 4 query blocks go into
                # ONE psum tile, so the G ring is touched once per head (its
                # slot frees only when the DVE reads it -- fewer allocations
                # mean fewer PE stalls on the DVE queue)
                h = 2 * ht + hh
                onat = onat_by_a[a]
                # 66-wide slots keep each bf16 psum write 4-byte aligned
                tr = gp.tile([128, 4 * 66], BF, tag="G", name="tr")
                for c in range(4):
                    nc.tensor.transpose(
                        tr[:, c * 66:c * 66 + HD + 1],
                        oT_sb[:, c * 128:(c + 1) * 128],
                        ident[:HD + 1, :HD + 1],
                    )
                recip = rcp.tile([128, 4], FP, tag="recip", name="recip")
                trv = tr[:, :].rearrange("p (c d) -> p c d", c=4)
                nc.vector.reciprocal(recip[:, :], trv[:, :, HD])
                for c in range(4):
                    nc.vector.tensor_scalar_mul(
                        onat[:, c * WC + h * 64:c * WC + (h + 1) * 64],
                        tr[:, c * 66:c * 66 + HD], recip[:, c:c + 1]
                    )

            # ---- phase 1: the minimum needed by stripe a=0 head pair 0 ----
            proj_qk_unit(wq_sb, 0, qT_sb, 0, 0)
            proj_qk_unit(wk_sb, 2, kT_sb, 0, 0)
            proj_v_unit(0)
            proj_v_unit(1)

            # remaining projection units are doled out between attention
            # units, scheduled (just) before their first consumer, keeping
            # the PE busy while ACT works through the exp stream
            def q_(t, qtr):
                return lambda: proj_qk_unit(wq_sb, 0, qT_sb, t, qtr)

            def k_(t, qtr):
                return lambda: proj_qk_unit(wk_sb, 2, kT_sb, t, qtr)

            def v_(c):
                return lambda: proj_v_unit(c)

            # placement: just-before-first-consumer deadlines, spread so
            # every region keeps the PE slightly ahead of the exp stream.
            # Stripes run 0,1,3,2: the trailing stripe-2 region (24 units)
            # then still has its own q/k projections left as PE filler,
            # where stripe 3 last would leave the PE starved (and the HAM
            # clock gate re-throttling) for its final 16 units.
            # NOTE: stripe 3 consumes ALL kT quarters and v chunks, so only
            # the stripe-2 q projections can be held back for the tail
            filler = {
                0: [v_(2)], 1: [v_(3)], 2: [q_(1, 0)], 3: [k_(1, 0)],
                4: [q_(0, 1)], 5: [k_(0, 1)], 6: [v_(4)], 7: [v_(5)],
                9: [v_(6)], 11: [v_(7)], 13: [q_(1, 1)], 15: [k_(1, 1)],
                17: [q_(0, 3)], 19: [k_(0, 3)], 21: [v_(8)], 23: [k_(0, 2)],
                25: [v_(9)], 26: [v_(10)], 28: [v_(11)], 30: [v_(12)],
                32: [v_(13)], 34: [v_(14)], 36: [v_(15)], 38: [q_(1, 3)],
                39: [k_(1, 3)], 44: [k_(1, 2)], 50: [q_(0, 2)],
                58: [q_(1, 2)],
            }

            onat_by_a = {}
            deferred = []          # finish/store closures fed into the stream

            # ---- phases 2+3: attention, software-pipelined ----
            uidx = 0
            for a in (0, 1, 3, 2):
                nchunks = 4 * a + 4
                if a not in onat_by_a:
                    onat_by_a[a] = onp.tile([128, 4 * WC], FP, tag="onat",
                                            name="onat")
                for ht in range(2):
                    oTs = [otp.tile([HD + 1, 512], mybir.dt.float32,
                                    tag="oT", name="oT") for _ in range(2)]
                    pend = []
                    for b in range(nchunks):
                        pend.append((b, score_exp_unit(a, ht, b)))
                        if len(pend) > LAG:
                            bb, pt = pend.pop(0)
                            av_unit(a, ht, bb, nchunks, oTs, pt)
                        for f in filler.get(uidx, ()):
                            f()
                        for _ in range(2 if len(deferred) > 6 else 1):
                            if deferred:
                                deferred.pop(0)()
                        uidx += 1
                    # the tail P @ v matmuls and the oT psum drain flow into
                    # the NEXT pair's units (via the priority end of the
                    # deferred queue): by then the exp stream has caught up,
                    # so the in-order PE queue never waits at pair boundaries
                    bb0, pt0 = pend.pop(0)
                    av_unit(a, ht, bb0, nchunks, oTs, pt0)

                    def tail_av(a_=a, ht_=ht, p_=tuple(pend), n_=nchunks,
                                o_=oTs):
                        for bb, pt in p_:
                            av_unit(a_, ht_, bb, n_, o_, pt)

                    def drain(a_=a, ht_=ht, o_=oTs):
                        for hh in range(2):
                            oT_sb = osb.tile([HD + 1, 512], BF, tag="oTsb",
                                             name="oTsb")
                            nc.vector.tensor_copy(oT_sb[:, :], o_[hh][:, :])
                            deferred.append(
                                (lambda hh_=hh, t_=oT_sb:
                                 finish_head(a_, ht_, hh_, t_)))

                        # store this head pair once its finishes have run
                        def store():
                            nc.sync.dma_start(
                                out[a_ * 512:(a_ + 1) * 512,
                                    ht_ * 128:(ht_ + 1) * 128].rearrange(
                                    "(c p) j -> p c j", p=128),
                                onat_by_a[a_][:, :].rearrange(
                                    "p (c j) -> p c j", c=4)[
                                    :, :, ht_ * 128:(ht_ + 1) * 128],
                            )
                        deferred.append(store)

                    deferred.insert(0, drain)
                    deferred.insert(0, tail_av)
            while deferred:
                deferred.pop(0)()

    nc.compile()
    return nc


def _get_nc():
    global _NC
    if _NC is None:
        _NC = _build()
    return _NC


def make_in_maps(hidden_states, Wqk, bqk, Wv, bv):
    from ml_dtypes import bfloat16

    x = np.asarray(hidden_states, dtype=np.float32)
    Wqk = np.asarray(Wqk, dtype=np.float32)
    bqk = np.asarray(bqk, dtype=np.float32)
    Wv = np.asarray(Wv, dtype=np.float32)
    bv = np.asarray(bv, dtype=np.float32)

    def pack(w):
        # [1024, C] -> SBUF image [128, 8*C] (k-chunk-major columns)
        c = w.shape[1]
        return np.ascontiguousarray(
            w.reshape(NK, 128, c).transpose(1, 0, 2).reshape(128, NK * c)
        ).astype(bfloat16)

    ident = np.eye(128, dtype=bfloat16)
    tri = np.triu(np.ones((128, 128), np.float32)).astype(bfloat16)
    # x quarters as SBUF images: xq[q][p, k*512+j] = x[b].T[k*128+p, q*512+j]
    xqs = []
    for b in range(B):
        xT = x[b].T.reshape(NK, 128, 4, 512)
        xqs.append([np.ascontiguousarray(
            xT[:, :, q, :].transpose(1, 0, 2).reshape(128, NK * 512)
        ).astype(bfloat16) for q in range(4)])
    in_maps = []
    for c in range(8):
        b, ho = c // 4, (c % 4) * NHL
        cols = slice(ho * HD, (ho + NHL) * HD)
        wv_aug = np.zeros((HID, VC), np.float32)
        wvl = np.zeros((1, VC), np.float32)
        for h in range(NHL):
            wv_aug[:, h * 65:h * 65 + HD] = Wv[:, (ho + h) * HD:(ho + h + 1) * HD]
            wvl[0, h * 65:h * 65 + HD] = bv[(ho + h) * HD:(ho + h + 1) * HD]
            wvl[0, h * 65 + HD] = 1.0
        bqk_c = np.stack([bqk[:HID][cols][:128], bqk[:HID][cols][128:],
                          bqk[HID:][cols][:128], bqk[HID:][cols][128:]],
                         axis=1)
        m = {
            "wq": pack(Wqk[:, cols]),
            "wk": pack(Wqk[:, HID:][:, cols]),
            "wv": pack(wv_aug),
            "wvl": wvl.astype(bfloat16),
            "bqk": np.ascontiguousarray(bqk_c.astype(np.float32)),
            "ident": ident,
            "tri": tri,
        }
        for q in range(4):
            m[f"xq{q}"] = xqs[b][q]
        in_maps.append(m)
    return in_maps


def kernel(hidden_states, Wqk, bqk, Wv, bv):
    import time

    from concourse.bass_utils import run_bass_kernel_spmd

    in_maps = make_in_maps(hidden_states, Wqk, bqk, Wv, bv)
    res = None
    for attempt in range(3):
        try:
            res = run_bass_kernel_spmd(_get_nc(), in_maps, list(range(8)))
            break
        except Exception:
            # transient NRT_EXEC_UNIT_UNRECOVERABLE errors have been observed
            # on this fabric; back off and retry
            if attempt == 2:
                raise
            time.sleep(2.0)
    outp = np.empty((B, S, NH * HD), np.float32)
    for c in range(8):
        b, ho = c // 4, (c % 4) * NHL
        outp[b, :, ho * HD:(ho + NHL) * HD] = res.results[c]["out"]
    return outp


# revision 34
# speedup vs baseline: 1.0427x; 1.0169x over previous
"""Causal self-attention (B=2, S=2048, HID=1024, 16 heads x 64) on 8 trn2
NeuronCores.

Sharding: data-parallel over batch (cores 0-3 -> batch 0, cores 4-7 ->
batch 1), tensor-parallel over heads (4 heads per core via Wqk/Wv column
slices). Each core computes its 4 heads end-to-end; the [S, S] score
matrix stays core-local.

Per-core layout choices:
  - All matmul operands are bf16 (inputs are cast host-side): the PE
    streams 1 col/cycle at 2.4 GHz and FWL halves LDWEIGHTS time; fp32
    paths measured ~2x slower on HW. PSUM accumulation stays fp32.
  - q, k are produced TRANSPOSED ([head_cols, S]) so score matmuls need
    no on-device transposes; scores are computed transposed ([sk, sq])
    so the P @ v matmul consumes exp(scores) directly from SBUF.
  - v carries an appended ones-column per head; the attention output
    matmul then yields softmax row-sums in an extra partition row for
    free (no max-subtraction is needed: scores are O(5) so exp is safe
    in fp32, and masked entries are zeroed multiplicatively post-exp
    with a DVE multiply against a [128,128] triangle mask input).
  - Heads are processed in pairs: the two K=64 score matmuls sit in PE
    row-groups 0-63 / 64-127 and run concurrently in the array.
  - The P @ v matmuls run 2 chunks BEHIND the score/exp stream, so the
    in-order PE queue never stalls on the ACT engine's exp latency.
  - Inputs arrive host-prepacked in the exact SBUF image as ~13 large
    contiguous DMAs split across the two HWDGE issue queues (Sync +
    Scalar; each dma_start costs ~0.6-1us of issue time, and the front
    is aggregate-HBM-bandwidth-bound, so critical tiles issue first).
  - The triangle masking runs on GPSIMD (IRAM preloaded at t=0), off
    the exp -> P @ v critical path, keeping the DVE queue short: DVE
    reads are what free PSUM tiles for the next score matmuls.
  - Stripes run 0,1,3,2 so the trailing region still has projection
    work as PE filler; pair-tail P @ v + PSUM drain + head finalization
    (transpose + normalize + store) are deferred into the following
    units via a priority queue. With a scratch-tile warmup burst
    bridging the input DMA, the PE holds the HAM clock gate at 8/8 from
    ~14us to the tail (measured: one continuous 123us warm window).
"""
import sys

for _p in ("/opt/trn_rl_repo",):
    if _p not in sys.path:
        sys.path.insert(0, _p)

import numpy as np

B, S, HID = 2, 2048, 1024
NH, HD = 16, 64
NHL = 4            # heads per core
WC = NHL * HD      # 256 local q/k weight cols
VC = NHL * (HD + 1)  # 260 local v cols incl. ones col
NT = S // 128      # 16 key chunks
NA = S // 512      # 4 query stripes
NK = HID // 128    # 8 contraction chunks
LAG = 2            # P @ v trails the score/exp stream by this many chunks

_NC = None


def _build():
    from concourse import bacc, mybir
    from concourse.tile import TileContext

    FP = mybir.dt.float32
    BF = mybir.dt.bfloat16
    Exp = mybir.ActivationFunctionType.Exp

    nc = bacc.Bacc("TRN2", target_bir_lowering=False, debug=False, num_devices=8)

    # all inputs are host-prepacked into the exact SBUF image, so every
    # input DMA is a fully-contiguous identity copy (8KB/partition runs)
    xq_d = [nc.dram_tensor(f"xq{q}", [128, NK * 512], BF, kind="ExternalInput")
            for q in range(4)]
    wq = nc.dram_tensor("wq", [128, NK * WC], BF, kind="ExternalInput")
    wk = nc.dram_tensor("wk", [128, NK * WC], BF, kind="ExternalInput")
    wv = nc.dram_tensor("wv", [128, NK * VC], BF, kind="ExternalInput")
    wvl_d = nc.dram_tensor("wvl", [1, VC], BF, kind="ExternalInput")
    bqk = nc.dram_tensor("bqk", [128, 4], FP, kind="ExternalInput")
    ident_d = nc.dram_tensor("ident", [128, 128], BF, kind="ExternalInput")
    tri_d = nc.dram_tensor("tri", [128, 128], BF, kind="ExternalInput")
    out = nc.dram_tensor("out", [S, WC], FP, kind="ExternalOutput")

    with TileContext(nc) as tc:
        with (
            tc.tile_pool(name="inp", bufs=1) as inp,
            tc.tile_pool(name="ptp", bufs=4) as ptp,
            tc.tile_pool(name="osb", bufs=8) as osb,
            tc.tile_pool(name="rcp", bufs=4) as rcp,
            tc.tile_pool(name="onat", bufs=4) as onp,
            tc.tile_pool(name="G", bufs=3, space="PSUM") as gp,
            tc.tile_pool(name="oT", bufs=2, space="PSUM") as otp,
        ):
            # PE warmup on a zeroed scratch tile (no DMA dependency, so it
            # starts right after the engine preambles): keeps the PE busy
            # through the HAM SHORT window while the input DMA streams, so
            # the projection stream starts at 2.4 GHz instead of 1.2
            scratch = inp.tile([128, 512], BF, name="scratch")
            nc.vector.memset(scratch[:, :], 0.0)
            # preload the GPSIMD ucode IRAM (~6us, hidden in the preamble)
            # so the first in-stream gpsimd mask-multiply doesn't pay it
            gsc = inp.tile([128, 1], BF, name="gsc")
            nc.gpsimd.memset(gsc[:, :], 0.0)
            warm = gp.tile([128, 1024], mybir.dt.float32, tag="G", name="warm")
            for _ in range(32):
                nc.tensor.matmul(warm[:, :512], lhsT=scratch[:, :128],
                                 rhs=scratch[:, :], start=True, stop=True)

            # ---- persistent inputs in SBUF, contiguous identity DMAs
            # split across the two HWDGE issue queues (Sync + Scalar).
            # The front is HBM-bandwidth-bound: issue the critical-path
            # bytes (wq, x quarter 0, wk, wv) before the remaining x
            # quarters, which would otherwise steal bandwidth from them.
            ident = inp.tile([128, 128], BF, name="ident")
            nc.sync.dma_start(ident[:, :], ident_d[:, :])
            tri = inp.tile([128, 128], BF, name="tri")
            nc.scalar.dma_start(tri[:, :], tri_d[:, :])
            # x quarter 0 in two halves, one per issue queue (the front is
            # aggregate-HBM-bound: critical tiles first, big quarters last)
            xq = [[None, None] for _ in range(4)]
            wq_sb = inp.tile([128, NK * WC], BF, name="wq")
            nc.sync.dma_start(wq_sb[:, :], wq[:, :])
            xq[0][1] = inp.tile([128, 4 * 512], BF, name="xq0_1")
            nc.scalar.dma_start(xq[0][1][:, :], xq_d[0][:, 2048:4096])
            xq[0][0] = inp.tile([128, 4 * 512], BF, name="xq0_0")
            nc.sync.dma_start(xq[0][0][:, :], xq_d[0][:, 0:2048])
            wv_sb = inp.tile([128, NK * VC], BF, name="wv")
            nc.scalar.dma_start(wv_sb[:, :], wv[:, :])
            wk_sb = inp.tile([128, NK * WC], BF, name="wk")
            nc.sync.dma_start(wk_sb[:, :], wk[:, :])
            wv_last = inp.tile([1, VC], BF, name="wvl")
            nc.scalar.dma_start(wv_last[:, :], wvl_d[:, :])
            bqk_sb = inp.tile([128, 4], FP, name="bqk")
            nc.scalar.dma_start(bqk_sb[:, :], bqk[:, :])
            for qtr, eng in ((1, nc.scalar), (2, nc.sync), (3, nc.sync)):
                t = inp.tile([128, NK * 512], BF, name=f"xq{qtr}")
                eng.dma_start(t[:, :], xq_d[qtr][:, :])
                xq[qtr][0] = t

            def xk(k, qtr):
                if qtr == 0:
                    t = xq[0][k // 4]
                    return t[:, (k % 4) * 512:(k % 4 + 1) * 512]
                return xq[qtr][0][:, k * 512:(k + 1) * 512]

            # split by S-quarter so interleaved later-quarter projection
            # writes can't false-depend against earlier attention reads
            qT_sb = [[inp.tile([128, 512], BF, name=f"qT{t}_{n}")
                      for n in range(4)] for t in range(2)]
            kT_sb = [[inp.tile([128, 512], BF, name=f"kT{t}_{n}")
                      for n in range(4)] for t in range(2)]
            v_sb = [inp.tile([128, VC], BF, name=f"v{c}") for c in range(NT)]

            # ---- projection emitters ----
            def proj_qk_unit(wt, bcol, dst, t, qtr):
                g = gp.tile([128, 1024], mybir.dt.float32, tag="G", name="g")
                for k in range(NK):
                    nc.tensor.matmul(
                        g[:, :512],
                        lhsT=wt[:, k * WC + t * 128:k * WC + (t + 1) * 128],
                        rhs=xk(k, qtr),
                        start=(k == 0), stop=(k == NK - 1),
                    )
                nc.vector.tensor_scalar_add(
                    dst[t][qtr][:, :], g[:, :512], bqk_sb[:, bcol + t:bcol + t + 1]
                )

            def proj_v_unit(c):
                qtr, cc = divmod(c, 4)
                g = gp.tile([128, 1024], mybir.dt.float32, tag="G", name="g")
                for k in range(NK):
                    nc.tensor.matmul(
                        g[:, :VC],
                        lhsT=xk(k, qtr)[:, cc * 128:(cc + 1) * 128],
                        rhs=wv_sb[:, k * VC:(k + 1) * VC],
                        start=(k == 0), stop=False,
                    )
                nc.tensor.matmul(  # bias row + ones column (K=1)
                    # tri row 0 is all-ones: broadcasts wv_last to all rows
                    g[:, :VC], lhsT=tri[0:1, 0:128], rhs=wv_last[:, :],
                    start=False, stop=True,
                )
                nc.vector.tensor_copy(v_sb[c][:, :], g[:, :VC])

            # ---- attention emitters ----
            # score/exp for ONE key chunk b of a head PAIR:
            # g = [h0-slice | h1-slice], one exp covers both heads
            def score_exp_unit(a, ht, b):
                g = gp.tile([128, 1024], mybir.dt.float32, tag="G", name="g")
                kn, ko = divmod(b * 128, 512)
                # diagonal chunks: columns < off are fully masked -- skip
                # them in the score matmul, the exp, and the P @ v matmul
                off = max(0, (b - 4 * a) * 128)
                for hh in range(2):
                    hb = hh * 64
                    nc.tensor.matmul(
                        g[:, hh * 512 + off:(hh + 1) * 512],
                        lhsT=kT_sb[ht][kn][hb:hb + 64, ko:ko + 128],
                        rhs=qT_sb[ht][a][hb:hb + 64, off:],
                        start=True, stop=True,
                    )
                pt = ptp.tile([128, 1024], BF, tag="pt", name="pt")
                if off:
                    gv = g[:, :].rearrange("p (h w) -> p h w", h=2)[:, :, off:]
                    pv = pt[:, :].rearrange("p (h w) -> p h w", h=2)[:, :, off:]
                    nc.scalar.activation(pv, gv, Exp, scale=HD ** -0.5)
                else:
                    nc.scalar.activation(pt[:, :], g[:, :], Exp, scale=HD ** -0.5)
                if b >= 4 * a:
                    # triangular boundary block: multiplicative mask. On
                    # GPSIMD (otherwise idle) to keep the DVE queue short --
                    # DVE reads are what free PSUM slots for the PE.
                    for hh in range(2):
                        c0 = hh * 512 + off
                        nc.gpsimd.tensor_mul(
                            pt[:, c0:c0 + 128], pt[:, c0:c0 + 128], tri[:, :]
                        )
                return pt

            def av_unit(a, ht, b, nchunks, oTs, pt):
                off = max(0, (b - 4 * a) * 128)
                for hh in range(2):
                    h = 2 * ht + hh
                    nc.tensor.matmul(
                        oTs[hh][:, off:],
                        lhsT=v_sb[b][:, h * 65:(h + 1) * 65],
                        rhs=pt[:, hh * 512 + off:(hh + 1) * 512],
                        start=(b == 0), stop=(b == nchunks - 1),
                    )

            def finish_head(a, ht, hh, oT_sb):
                # transpose + normalize one head: all 4 query blocks go into
                # ONE psum tile, so the G ring is touched once per head (its
                # slot frees only when the DVE reads it -- fewer allocations
                # mean fewer PE stalls on the DVE queue)
                h = 2 * ht + hh
                onat = onat_by_a[a]
                # 66-wide slots keep each bf16 psum write 4-byte aligned
                tr = gp.tile([128, 4 * 66], BF, tag="G", name="tr")
                for c in range(4):
                    nc.tensor.transpose(
                        tr[:, c * 66:c * 66 + HD + 1],
                        oT_sb[:, c * 128:(c + 1) * 128],
                        ident[:HD + 1, :HD + 1],
                    )
                recip = rcp.tile([128, 4], FP, tag="recip", name="recip")
                trv = tr[:, :].rearrange("p (c d) -> p c d", c=4)
                nc.vector.reciprocal(recip[:, :], trv[:, :, HD])
                for c in range(4):
                    nc.vector.tensor_scalar_mul(
                        onat[:, c * WC + h * 64:c * WC + (h + 1) * 64],
                        tr[:, c * 66:c * 66 + HD], recip[:, c:c + 1]
                    )

            # ---- phase 1: the minimum needed by stripe a=0 head pair 0.
            # v0/v1 ride the first filler slots instead: scores only need
            # q/k, so the exp stream starts earlier and builds ACT backlog
            proj_qk_unit(wq_sb, 0, qT_sb, 0, 0)
            proj_qk_unit(wk_sb, 2, kT_sb, 0, 0)

            # remaining projection units are doled out between attention
            # units, scheduled (just) before their first consumer, keeping
            # the PE busy while ACT works through the exp stream
            def q_(t, qtr):
                return lambda: proj_qk_unit(wq_sb, 0, qT_sb, t, qtr)

            def k_(t, qtr):
                return lambda: proj_qk_unit(wk_sb, 2, kT_sb, t, qtr)

            def v_(c):
                return lambda: proj_v_unit(c)

            # placement: just-before-first-consumer deadlines, spread so
            # every region keeps the PE slightly ahead of the exp stream.
            # Stripes run 0,1,3,2: the trailing stripe-2 region (24 units)
            # then still has its own q/k projections left as PE filler,
            # where stripe 3 last would leave the PE starved (and the HAM
            # clock gate re-throttling) for its final 16 units.
            # NOTE: stripe 3 consumes ALL kT quarters and v chunks, so only
            # the stripe-2 q projections can be held back for the tail
            filler = {
                0: [v_(0)], 1: [v_(1)], 2: [q_(1, 0), v_(2)],
                3: [k_(1, 0), v_(3)],
                4: [q_(0, 1)], 5: [k_(0, 1)], 6: [v_(4)], 7: [v_(5)],
                9: [v_(6)], 11: [v_(7)], 13: [q_(1, 1)], 15: [k_(1, 1)],
                17: [q_(0, 3)], 19: [k_(0, 3)], 21: [v_(8)], 23: [k_(0, 2)],
                25: [v_(9)], 26: [v_(10)], 28: [v_(11)], 30: [v_(12)],
                32: [v_(13)], 34: [v_(14)], 36: [v_(15)], 38: [q_(1, 3)],
                39: [k_(1, 3)], 44: [k_(1, 2)], 50: [q_(0, 2)],
                58: [q_(1, 2)],
            }

            onat_by_a = {}
            deferred = []          # finish/store closures fed into the stream

            # ---- phases 2+3: attention, software-pipelined ----
            uidx = 0
            for a in (0, 1, 3, 2):
                nchunks = 4 * a + 4
                if a not in onat_by_a:
                    onat_by_a[a] = onp.tile([128, 4 * WC], FP, tag="onat",
                                            name="onat")
                for ht in range(2):
                    oTs = [otp.tile([HD + 1, 512], mybir.dt.float32,
                                    tag="oT", name="oT") for _ in range(2)]
                    pend = []
                    for b in range(nchunks):
                        pend.append((b, score_exp_unit(a, ht, b)))
                        if len(pend) > LAG:
                            bb, pt = pend.pop(0)
                            av_unit(a, ht, bb, nchunks, oTs, pt)
                        for f in filler.get(uidx, ()):
                            f()
                        for _ in range(2 if len(deferred) > 6 else 1):
                            if deferred:
                                deferred.pop(0)()
                        uidx += 1
                    # the tail P @ v matmuls and the oT psum drain flow into
                    # the NEXT pair's units (via the priority end of the
                    # deferred queue): by then the exp stream has caught up,
                    # so the in-order PE queue never waits at pair boundaries
                    bb0, pt0 = pend.pop(0)
                    av_unit(a, ht, bb0, nchunks, oTs, pt0)

                    def tail_av(a_=a, ht_=ht, p_=tuple(pend), n_=nchunks,
                                o_=oTs):
                        for bb, pt in p_:
                            av_unit(a_, ht_, bb, n_, o_, pt)

                    def drain(a_=a, ht_=ht, o_=oTs):
                        for hh in range(2):
                            oT_sb = osb.tile([HD + 1, 512], BF, tag="oTsb",
                                             name="oTsb")
                            nc.vector.tensor_copy(oT_sb[:, :], o_[hh][:, :])
                            deferred.append(
                                (lambda hh_=hh, t_=oT_sb:
                                 finish_head(a_, ht_, hh_, t_)))

                        # store this head pair once its finishes have run
                        def store():
                            nc.sync.dma_start(
                                out[a_ * 512:(a_ + 1) * 512,
                                    ht_ * 128:(ht_ + 1) * 128].rearrange(
                                    "(c p) j -> p c j", p=128),
                                onat_by_a[a_][:, :].rearrange(
                                    "p (c j) -> p c j", c=4)[
                                    :, :, ht_ * 128:(ht_ + 1) * 128],
                            )
                        deferred.append(store)

                    deferred.insert(0, drain)
                    deferred.insert(0, tail_av)
            while deferred:
                deferred.pop(0)()

    nc.compile()
    return nc


def _get_nc():
    global _NC
    if _NC is None:
        _NC = _build()
    return _NC


def make_in_maps(hidden_states, Wqk, bqk, Wv, bv):
    from ml_dtypes import bfloat16

    x = np.asarray(hidden_states, dtype=np.float32)
    Wqk = np.asarray(Wqk, dtype=np.float32)
    bqk = np.asarray(bqk, dtype=np.float32)
    Wv = np.asarray(Wv, dtype=np.float32)
    bv = np.asarray(bv, dtype=np.float32)

    def pack(w):
        # [1024, C] -> SBUF image [128, 8*C] (k-chunk-major columns)
        c = w.shape[1]
        return np.ascontiguousarray(
            w.reshape(NK, 128, c).transpose(1, 0, 2).reshape(128, NK * c)
        ).astype(bfloat16)

    ident = np.eye(128, dtype=bfloat16)
    tri = np.triu(np.ones((128, 128), np.float32)).astype(bfloat16)
    # x quarters as SBUF images: xq[q][p, k*512+j] = x[b].T[k*128+p, q*512+j]
    xqs = []
    for b in range(B):
        xT = x[b].T.reshape(NK, 128, 4, 512)
        xqs.append([np.ascontiguousarray(
            xT[:, :, q, :].transpose(1, 0, 2).reshape(128, NK * 512)
        ).astype(bfloat16) for q in range(4)])
    in_maps = []
    for c in range(8):
        b, ho = c // 4, (c % 4) * NHL
        cols = slice(ho * HD, (ho + NHL) * HD)
        wv_aug = np.zeros((HID, VC), np.float32)
        wvl = np.zeros((1, VC), np.float32)
        for h in range(NHL):
            wv_aug[:, h * 65:h * 65 + HD] = Wv[:, (ho + h) * HD:(ho + h + 1) * HD]
            wvl[0, h * 65:h * 65 + HD] = bv[(ho + h) * HD:(ho + h + 1) * HD]
            wvl[0, h * 65 + HD] = 1.0
        bqk_c = np.stack([bqk[:HID][cols][:128], bqk[:HID][cols][128:],
                          bqk[HID:][cols][:128], bqk[HID:][cols][128:]],
                         axis=1)
        m = {
            "wq": pack(Wqk[:, cols]),
            "wk": pack(Wqk[:, HID:][:, cols]),
            "wv": pack(wv_aug),
            "wvl": wvl.astype(bfloat16),
            "bqk": np.ascontiguousarray(bqk_c.astype(np.float32)),
            "ident": ident,
            "tri": tri,
        }
        for q in range(4):
            m[f"xq{q}"] = xqs[b][q]
        in_maps.append(m)
    return in_maps


def kernel(hidden_states, Wqk, bqk, Wv, bv):
    import time

    from concourse.bass_utils import run_bass_kernel_spmd

    in_maps = make_in_maps(hidden_states, Wqk, bqk, Wv, bv)
    res = None
    for attempt in range(3):
        try:
            res = run_bass_kernel_spmd(_get_nc(), in_maps, list(range(8)))
            break
        except Exception:
            # transient NRT_EXEC_UNIT_UNRECOVERABLE errors have been observed
            # on this fabric; back off and retry
            if attempt == 2:
                raise
            time.sleep(2.0)
    outp = np.empty((B, S, NH * HD), np.float32)
    for c in range(8):
        b, ho = c // 4, (c % 4) * NHL
        outp[b, :, ho * HD:(ho + NHL) * HD] = res.results[c]["out"]
    return outp
